# revision 71
# baseline (speedup 1.0000x reference)
"""Trainium2 Bass kernel for nn_CAGetBoard (neural CA step).

Takes FULL inputs, shards batch across 8 NeuronCores (pure data parallel),
runs a Bass/Tile kernel per core, gathers the FULL output.

Per-core pipeline (B/8 images each), all matmuls bf16, software-pipelined
ACROSS images (image b's finish interleaves image b+1's conv) so the
tensor engine never drains:
  - conv1 (Sobel folded into a 16->128 3x3 conv) = 2 accumulating matmuls
    (K=48 left tap + K=97 right/center + bias-ones row) over a 6-copy
    row/col-shifted stack; A copies at col+1, B copies at col+2 read the
    SAME full-width rows from a bf16 DRAM scratch (512B runs, no edge
    wrap); 258-stride rows + once-zeroed pad cols give SAME padding;
    b1 rides a 97th all-ones stack row so relu drains need no bias and
    can split 5:3 between ACT and DVE.
  - mm2 (128->16) col-tiled x4 (M=32, dup weights) -> tanh(+b2) ACT drain
    into per-PAIR [128,4096] d tiles (two 32-row blocks per tile).
  - alive masks in row layout; 3x3 binary dilation via banded bf16 matmuls.
  - finishing per block-pair, bf16 on DVE (2x mode): t=d*u16; t+=x via 4
    SWDGE accum DMAs (real rows only); one tensor_scalar with per-channel
    [128,1] bounds fuses the ch0-2 clamp; t*=a16; 4 SWDGE cast DMAs emit
    the f32 output. u/alive masks reach the packed layout via small
    row->[4,4096] reshape DMAs + one 0-stride broadcast DMA each.
  - last image's pair-finishes are split into alive-independent /
    alive-dependent halves to shorten the pipeline tail.
"""

import numpy as np

import concourse.bass as bass
import concourse.bacc as bacc
import concourse.tile as tile
import concourse.mybir as mybir
from concourse.ap import AP as RawAP
from concourse.bass_utils import run_bass_kernel_spmd

dt = mybir.dt
F32 = dt.float32
BF16 = dt.bfloat16
AF = mybir.ActivationFunctionType
OP = mybir.AluOpType

N_CORES = 8
C = 16
H = 256
W = 256
TR = 32                    # rows per compute block
WS = W + 2                 # padded row stride
N_BLK = H // TR
N_GRP = 4                  # mm2 groups per block (2048 px each)
PX_IMG = H * W
CHW = C * H * W
PADE = 512                 # bf16 scratch pad elements on each side
EPS = 0.5
ALIVE_T = 0.1
BIG = 60000.0

# which relu drains go to DVE instead of ACT (index 2*g+ip in 0..7)
DVE_DRAINS = (1, 4, 6)


def _build_consts(w1, b1, w2, b2):
    w1 = np.asarray(w1, np.float32)
    w2 = np.asarray(w2, np.float32)
    b1 = np.asarray(b1, np.float32)
    sob = np.array([[-1., 0., 1.], [-2., 0., 2.], [-1., 0., 1.]], np.float32)
    W1x, W1gx, W1gy = w1[:, 0:16], w1[:, 16:32], w1[:, 32:48]
    k1f = (W1gx[:, :, None, None] * sob[None, None, :, :]
           + W1gy[:, :, None, None] * sob.T[None, None, :, :])
    k1f[:, :, 1, 1] += W1x
    lhs = np.transpose(k1f, (3, 2, 1, 0)).reshape(3, 48, 128)
    lhsA = lhs[0].copy()
    # A-groups (written at col+1) deliver the RIGHT tap under slice [2:W+2];
    # B-groups (written at col+2) deliver CENTER; bias-ones row last.
    lhsB = np.concatenate([lhs[2], lhs[1], b1.reshape(1, 128)], axis=0)

    w2dup = np.zeros((128, 32), np.float32)
    w2dup[:, 0:16] = w2.T
    w2dup[:, 16:32] = w2.T

    b2dup = np.zeros((128, 1), np.float32)
    for i in range(4):
        for d in range(2):
            s = 32 * i + 16 * d
            b2dup[s:s + 16, 0] = b2

    bandB = np.zeros((128, 128), np.float32)
    for k in range(128):
        bandB[k, max(0, k - 1):k + 2] = 1.0
    bandClo = np.zeros((128, 128), np.float32)
    bandClo[0, 127] = 1.0
    bandChi = np.zeros((128, 128), np.float32)
    bandChi[127, 0] = 1.0
    clo1 = np.zeros((1, 128), np.float32)
    clo1[0, 127] = 1.0

    ub = np.full((128, 1), BIG, np.float32)
    lb = np.full((128, 1), -BIG, np.float32)
    for i in range(4):
        for d in range(2):
            s = 32 * i + 16 * d
            ub[s:s + 3, 0] = 1.0
            lb[s:s + 3, 0] = 0.0

    return dict(
        lhsA=lhsA, lhsB=lhsB,
        w2dup=w2dup, b2dup=b2dup,
        bandB=bandB, bandClo=bandClo, bandChi=bandChi, clo1=clo1,
        ub=ub, lb=lb,
    )


CONST_SPECS = dict(
    lhsA=([48, 128], BF16), lhsB=([97, 128], BF16),
    w2dup=([128, 32], BF16), b2dup=([128, 1], F32),
    bandB=([128, 128], BF16), bandClo=([128, 128], BF16),
    bandChi=([128, 128], BF16), clo1=([1, 128], BF16),
    ub=([128, 1], F32), lb=([128, 1], F32),
)


def build_program(n_img, reps=1):
    nc = bacc.Bacc("TRN2", target_bir_lowering=False)

    x_d = nc.dram_tensor("x", [n_img, C, H, W], F32, kind="ExternalInput")
    rand_d = nc.dram_tensor("rand", [n_img, H, W], F32, kind="ExternalInput")
    cst_d = {k: nc.dram_tensor(k, sh, d, kind="ExternalInput")
             for k, (sh, d) in CONST_SPECS.items()}
    out_d = nc.dram_tensor("out", [n_img, C, H, W], F32, kind="ExternalOutput")
    alive_d = nc.dram_tensor("alivescr", [n_img, PX_IMG], BF16, kind="Internal")
    xbf_d = nc.dram_tensor("xbfscr", [2 * PADE + n_img * CHW], BF16,
                           kind="Internal")

    xf = x_d.ap().rearrange("b c h w -> b c (h w)")
    outf = out_d.ap().rearrange("b c h w -> b c (h w)")
    randf = rand_d.ap().rearrange("b h w -> b (h w)")

    with tile.TileContext(nc) as tc:
        _emit(nc, tc, n_img, xf, randf, cst_d, outf, alive_d.ap(),
              xbf_d, reps)
    nc.compile()
    return nc


def _emit(nc, tc, n_img, xf, randf, cst_d, outf, alivef, xbf_d, reps=1):
    from contextlib import ExitStack
    ctx = ExitStack()

    def pool(name, bufs, **kw):
        return ctx.enter_context(tc.tile_pool(name=name, bufs=bufs, **kw))

    consts = pool("consts", 1)
    stackp = pool("stack", 1)
    hgrp_p = pool("hgrp", 4)
    dgrp_p = pool("dgrp", 5)
    fin_p = pool("fin", 2)
    row_p = pool("rows", 4)
    rowsm_p = pool("rowsm", 2)
    u32_p = pool("u32", 2)
    misc_p = pool("misc", 1)
    conv_ps = pool("convps", 2, space="PSUM")
    mask_ps = pool("maskps", 1, space="PSUM")
    mm2_ps = pool("mm2ps", 3, space="PSUM")

    cst = {}
    early = ("lhsA", "lhsB", "w2dup", "b2dup")
    for k in early:
        sh, d = CONST_SPECS[k]
        t = consts.tile(sh, d, tag=k, name=k)
        nc.sync.dma_start(t[:], cst_d[k].ap())
        cst[k] = t
    for k, (sh, d) in CONST_SPECS.items():
        if k in early:
            continue
        t = consts.tile(sh, d, tag=k, name=k)
        nc.scalar.dma_start(t[:], cst_d[k].ap())
        cst[k] = t

    # flat bf16 x scratch views
    xbf_flat = xbf_d.ap()
    xbf_img = xbf_flat[PADE:PADE + n_img * CHW].rearrange(
        "(b c p) -> b c p", b=n_img, c=C)

    # stack tiles: [97, TR*WS]; partitions 0-95 = 6 shifted copies
    # (a=col-shift 2, b=row-shift 3, c=chan 16), partition 96 = bias ones.
    stacks = []
    for s in range(3):
        st = stackp.tile([97, TR * WS], BF16, tag=f"stack{s}",
                         name=f"stack{s}")
        st3 = st.rearrange("p (r j) -> p r j", j=WS)
        nc.vector.memset(st3[0:96, :, 0:1], 0.0)           # pad col 0
        nc.vector.memset(st3[0:96, :, WS - 1:WS], 0.0)     # pad col W+1
        nc.vector.memset(st[96:97, :], 1.0)
        stacks.append(st3)

    sdil = []
    for s in range(4):
        t = misc_p.tile([128, WS], F32, tag=f"sdil{s}", name=f"sdil{s}")
        nc.vector.memset(t[:, 0:1], 0.0)
        nc.vector.memset(t[:, W + 1:W + 2], 0.0)
        sdil.append(t)

    def dilate_half(half, b_main, extra_lhs, extra_rhs, out_t, sgrp=0):
        vs = mask_ps.tile([128, W], F32, tag="mask", name="vs")
        nc.tensor.matmul(vs[:], cst["bandB"][:], b_main[:],
                         start=True, stop=(extra_lhs is None))
        if extra_lhs is not None:
            nc.tensor.matmul(vs[:], extra_lhs, extra_rhs,
                             start=False, stop=True)
        s = sdil[2 * sgrp + half]
        nc.scalar.activation(s[:, 1:W + 1], vs[:], AF.Copy)
        t = rowsm_p.tile([128, W], F32, tag="dil_t", name="dil_t")
        nc.vector.tensor_add(t[:], s[:, 0:W], s[:, 2:W + 2])
        nc.vector.tensor_add(t[:], t[:], s[:, 1:W + 1])
        nc.vector.tensor_single_scalar(out_t[:], t[:], 0.5, OP.is_gt)

    # per-image state
    st_x3row = [None] * n_img
    st_randrow = [None] * n_img
    st_prealive = [None] * n_img
    st_d3row = [None] * n_img
    st_bpost0 = [None] * n_img
    dkeep = {}
    stack_ctr = [0]

    def cast_x(b):
        # f32 -> bf16 cast of one image into the padded flat scratch, 2 DMAs
        for half in range(2):
            dst = xbf_img[b][:, half * (PX_IMG // 2):(half + 1) * (PX_IMG // 2)]
            nc.gpsimd.dma_start(
                dst, xf[b][:, half * (PX_IMG // 2):(half + 1) * (PX_IMG // 2)])

    def prepass(b):
        x3row, randrow, bpre, prealive = [], [], [], []
        for half in range(2):
            xt = row_p.tile([128, W], F32, tag="x3row", name="x3row")
            nc.sync.dma_start(
                xt[:], xf[b, 3, half * 128 * W:(half + 1) * 128 * W]
                .rearrange("(p w) -> p w", w=W))
            x3row.append(xt)
            rt = row_p.tile([128, W], F32, tag="randrow", name="randrow")
            nc.sync.dma_start(
                rt[:], randf[b, half * 128 * W:(half + 1) * 128 * W]
                .rearrange("(p w) -> p w", w=W))
            randrow.append(rt)
            bt = row_p.tile([128, W], BF16, tag="bpre", name="bpre")
            nc.vector.tensor_single_scalar(bt[:], xt[:], ALIVE_T, OP.is_gt)
            bpre.append(bt)
            prealive.append(row_p.tile([128, W], BF16, tag="prealive",
                                       name="prealive"))
        dilate_half(0, bpre[0], cst["bandClo"][:], bpre[1][:], prealive[0])
        dilate_half(1, bpre[1], cst["bandChi"][:], bpre[0][:], prealive[1])
        st_x3row[b] = x3row
        st_randrow[b] = randrow
        st_prealive[b] = prealive
        st_d3row[b] = [row_p.tile([128, W], BF16, tag="d3row", name="d3row")
                       for _ in range(2)]

    def compute_block(b, blk):
        r0 = blk * TR
        if blk % 2 == 0:
            dgp = dgrp_p.tile([128, 4096], BF16, tag="d", name="d")
            dkeep[(b, blk // 2)] = dgp
        else:
            dgp = dkeep[(b, blk // 2)]
        dgb = dgp[:, 2048 * (blk % 2):2048 * (blk % 2) + 2048]
        st3 = stacks[stack_ctr[0] % 3]
        stack_ctr[0] += 1

        # six shifted copies: A groups (p 0:48) at col+1, B groups (p 48:96)
        # at col+2, identical full-width sources; v = row shift. Edge rows
        # are clipped out of the DMA and pre-zeroed (32-aligned memsets).
        off0 = PADE + b * CHW + (r0 - 1) * W
        if blk == 0:
            nc.vector.memset(st3[0:16, 0:1, :], 0.0)
            nc.vector.memset(st3[32:64, 0:1, :], 0.0)
        if blk == N_BLK - 1:
            nc.vector.memset(st3[32:48, TR - 1:TR, :], 0.0)
            nc.vector.memset(st3[64:96, TR - 1:TR, :], 0.0)
        for a in range(2):
            for v in range(3):
                rlo = 1 if (blk == 0 and v == 0) else 0
                rhi = TR - 1 if (blk == N_BLK - 1 and v == 2) else TR
                p0 = 48 * a + 16 * v
                src = RawAP(xbf_flat.tensor, off0 + (v + rlo) * W,
                            [[H * W, C], [W, rhi - rlo], [1, W]])
                eng = (nc.sync, nc.scalar)[(3 * a + v) % 2]
                eng.dma_start(
                    st3[p0:p0 + 16, rlo:rhi, 1 + a:1 + a + W], src)

        for g in range(N_GRP):
            hg = hgrp_p.tile([128, 2048], BF16, tag="hgrp", name="hgrp")
            for ip in range(2):
                acc = conv_ps.tile([128, 1024], F32, tag="conv", name="conv")
                for i in (2 * ip, 2 * ip + 1):
                    chk = 4 * i + g
                    asl = acc[:, 512 * (i - 2 * ip):512 * (i - 2 * ip + 1)]
                    nc.tensor.matmul(
                        asl, cst["lhsA"][:],
                        st3[0:48, 2 * chk:2 * chk + 2, 0:W],
                        start=True, stop=False)
                    nc.tensor.matmul(
                        asl, cst["lhsB"][:],
                        st3[0:97, 2 * chk:2 * chk + 2, 2:W + 2],
                        start=False, stop=True)
                hsl = hg[:, 1024 * ip:1024 * (ip + 1)]
                if (2 * g + ip) in DVE_DRAINS:
                    nc.vector.tensor_single_scalar(hsl, acc[:], 0.0, OP.max)
                else:
                    nc.scalar.activation(hsl, acc[:], AF.Relu)
            mm = mm2_ps.tile([128, 512], F32, tag="mm2", name="mm2")
            for i in range(4):
                nc.tensor.matmul(
                    mm[32 * i:32 * i + 32, :],
                    cst["w2dup"][:],
                    hg[:, 512 * i:512 * (i + 1)],
                    start=True, stop=True,
                    tile_position=(0, 32 * i))
            nc.scalar.activation(dgb[:, 512 * g:512 * (g + 1)], mm[:],
                                 AF.Tanh, bias=cst["b2dup"][:, 0:1])

        # d channel-3 rows back to row layout (contiguous-partition sources)
        half = blk // 4
        rbase = (TR * blk) % 128
        for i in range(4):
            nc.scalar.dma_start(
                st_d3row[b][half][rbase + 8 * i:rbase + 8 * i + 8, :],
                dgb[32 * i + 3:32 * i + 4, :])

    def post_binary(rows_ap_rand, rows_ap_x3, d3_ap, out_t):
        m = rowsm_p.tile(list(out_t.shape), F32, tag="postm", name="postm")
        nc.vector.scalar_tensor_tensor(
            m[:], rows_ap_rand, EPS, d3_ap, op0=OP.is_lt, op1=OP.mult)
        nc.vector.tensor_add(m[:], m[:], rows_ap_x3)
        nc.vector.tensor_single_scalar(out_t[:], m[:], ALIVE_T, OP.is_gt)

    def alive_store(b, half, ar):
        nc.sync.dma_start(
            alivef[b, half * 128 * W:(half + 1) * 128 * W]
            .rearrange("(p w) -> p w", w=W), ar[:])

    def alive0(b):
        bpost0 = rowsm_p.tile([128, W], BF16, tag="bpost0", name="bpost0")
        post_binary(st_randrow[b][0][:], st_x3row[b][0][:],
                    st_d3row[b][0][:], bpost0)
        bp128 = rowsm_p.tile([1, W], BF16, tag="bp128", name="bp128")
        post_binary(st_randrow[b][1][0:1, :], st_x3row[b][1][0:1, :],
                    st_d3row[b][1][0:1, :], bp128)
        postal0 = rowsm_p.tile([128, W], BF16, tag="postal0", name="postal0")
        dilate_half(0, bpost0, cst["clo1"][:], bp128[:], postal0, sgrp=1)
        ar0 = rowsm_p.tile([128, W], BF16, tag="ar0", name="ar0")
        nc.vector.tensor_mul(ar0[:], st_prealive[b][0][:], postal0[:])
        alive_store(b, 0, ar0)
        st_bpost0[b] = bpost0

    def alive1(b):
        bpost1 = rowsm_p.tile([128, W], BF16, tag="bpost1", name="bpost1")
        post_binary(st_randrow[b][1][:], st_x3row[b][1][:],
                    st_d3row[b][1][:], bpost1)
        postal1 = rowsm_p.tile([128, W], BF16, tag="postal1", name="postal1")
        dilate_half(1, bpost1, cst["bandChi"][:], st_bpost0[b][:], postal1,
                    sgrp=1)
        ar1 = rowsm_p.tile([128, W], BF16, tag="ar1", name="ar1")
        nc.vector.tensor_mul(ar1[:], st_prealive[b][1][:], postal1[:])
        alive_store(b, 1, ar1)

    st_ft = {}

    def fp_early(b, k):
        # alive-independent part: t = clip(d*u + x)
        pxp = 2 * k * TR * W
        half = k // 2
        rb = (64 * k) % 128
        u32 = u32_p.tile([64, W], BF16, tag="u32", name="u32")
        nc.vector.tensor_single_scalar(
            u32[:], st_randrow[b][half][rb:rb + 64, :], EPS, OP.is_lt)
        # reshape to [4, 4096]: partition i holds pixels 2048i..+2048 of
        # each block (kb column halves)
        u4 = u32_p.tile([4, 4096], BF16, tag="u4", name="u4")
        for kb in range(2):
            nc.sync.dma_start(
                u4[:, 2048 * kb:2048 * (kb + 1)]
                .rearrange("i (r w) -> i r w", w=W),
                u32[32 * kb:32 * kb + 32, :])
        u16 = fin_p.tile([128, 4096], BF16, tag="u16", name="u16")
        nc.sync.dma_start(
            u16[:], u4[:].unsqueeze(1).broadcast_to([4, 32, 4096]))

        dgp = dkeep.pop((b, k))
        t = fin_p.tile([128, 4096], BF16, tag="ft", name="ft")
        st_ft[(b, k)] = t
        nc.vector.tensor_mul(t[:], dgp[:], u16[:])
        # t += x (bf16) via SWDGE accumulate DMAs, real (non-dup) rows only
        for i in range(4):
            src = RawAP(xbf_flat.tensor, PADE + b * CHW + pxp + 2048 * i,
                        [[H * W, C], [8192, 2], [1, 2048]])
            nc.gpsimd.dma_start(t[32 * i:32 * i + 16, :], src,
                                accum_op=OP.add)
        nc.vector.tensor_scalar(t[:], t[:], cst["ub"][:, 0:1],
                                cst["lb"][:, 0:1], op0=OP.min, op1=OP.max)

    def fp_late(b, k):
        # alive-dependent tail: t *= a16; cast-DMA the f32 output pair
        pxp = 2 * k * TR * W
        a16 = fin_p.tile([128, 4096], BF16, tag="a16", name="a16")
        for kb in range(2):
            asrc = (alivef[b, pxp + 8192 * kb:pxp + 8192 * kb + 8192]
                    .rearrange("(i n) -> i n", n=2048)
                    .unsqueeze(1).broadcast_to([4, 32, 2048]))
            nc.scalar.dma_start(a16[:, 2048 * kb:2048 * (kb + 1)], asrc)
        t = st_ft.pop((b, k))
        nc.vector.tensor_mul(t[:], t[:], a16[:])
        for i in range(4):
            dst = RawAP(outf.tensor, b * CHW + pxp + 2048 * i,
                        [[H * W, C], [8192, 2], [1, 2048]])
            nc.gpsimd.dma_start(dst, t[32 * i:32 * i + 16, :])

    def finish_pair(b, k):
        fp_early(b, k)
        fp_late(b, k)

    # ---------------- schedule ----------------
    for rep in range(reps):
        for b in range(n_img):
            cast_x(b)
        prepass(0)
        for blk in range(5):
            compute_block(0, blk)
        alive0(0)
        for b in range(n_img):
            compute_block(b, 5)
            finish_pair(b, 0)
            compute_block(b, 6)
            compute_block(b, 7)
            finish_pair(b, 1)
            if b + 1 == n_img:
                fp_early(b, 2)
                fp_early(b, 3)
            alive1(b)
            nb = b + 1
            if nb < n_img:
                prepass(nb)
                compute_block(nb, 0)
                finish_pair(b, 2)
                compute_block(nb, 1)
                finish_pair(b, 3)
                compute_block(nb, 2)
                compute_block(nb, 3)
                compute_block(nb, 4)
                alive0(nb)
            else:
                fp_late(b, 2)
                fp_late(b, 3)

    ctx.close()


# ---------------------------------------------------------------------------

_NC_CACHE = {}


def _get_nc(n_img, reps=1):
    key = (n_img, reps)
    if key not in _NC_CACHE:
        _NC_CACHE[key] = build_program(n_img, reps)
    return _NC_CACHE[key]


def kernel(x, w1, b1, w2, b2, rand_mask):
    x = np.ascontiguousarray(np.asarray(x, np.float32))
    rand_mask = np.ascontiguousarray(np.asarray(rand_mask, np.float32))
    B = x.shape[0]
    n_img = B // N_CORES
    consts = _build_consts(w1, b1, w2, b2)
    cast = {k: np.ascontiguousarray(v.astype(mybir.dt.np(CONST_SPECS[k][1])))
            for k, v in consts.items()}

    nc = _get_nc(n_img)
    in_maps = []
    for k in range(N_CORES):
        sl = slice(k * n_img, (k + 1) * n_img)
        in_maps.append(dict(x=x[sl], rand=rand_mask[sl, 0], **cast))
    res = run_bass_kernel_spmd(nc, in_maps, core_ids=list(range(N_CORES)))
    out = np.concatenate([res.results[k]["out"] for k in range(N_CORES)],
                         axis=0)
    return out.astype(np.float32)
